# revision 60
# baseline (speedup 1.0000x reference)
"""Euler characteristic curve (cubical complex) kernel for Trainium2.

Problem: x [32,16,128,128] f32 -> ECC [32,16,64] f32.
Per (b,c) slice: every cell of the 255x255 vertex-mode cubical grid has
filtration value = max over its corner vertices, bin K = ceil(63*x) in [0,63],
ECC(t) = #V(K<=t) - #Eh(K<=t) - #Ev(K<=t) + #Q(K<=t)  (cumulative counts).

Strategy (per core, 64 slices, pure data parallel over 8 cores):
 - single group: each of the 128 partitions holds 64 owned image rows plus an
   "above" aux row and a "below" aux row (66 x 128 f32).  A slice (128 rows)
   spans 2 partitions; aux rows crossing a slice boundary get 2.0 phantoms
   (bin 126: dominates every max, loses every >= tie, never counted).
 - exact binning: y=63*x, Ki=int cast, K = cast_back(Ki) + (y > cast_back);
   exact ceil under either truncating or round-to-nearest cast semantics.
 - cell bins via bf16 neighbor maxes (x->63x->ceil monotone: bin(max)=max(bins)).
 - kcat bf16 layout [V | Q | Eh | Ev | kaux | ehaux | kbaux]: pos cell block
   [V|Q] = [0:16384), neg block [Eh|Ev] = [16384:32768).
 - lower-star vertex weights (DVE, bf16, mostly 2x mode): every cell is
   assigned to the corner vertex attaining its max (ties: prefer larger row,
   then larger column), giving integer vertex weights w in [-3,1] with
   chi(t) = sum_v w_v * [K_v <= t] exactly.  Validated vs brute force in
   numpy (including the 2-partition slice split with aux rows).
 - threshold loop split across engines, fully overlapped:
   * DVE t in [0,TSPLIT): ONE fused scalar_tensor_tensor pass over the 8192
     vertices: (K is_le t) * w, accum_out = per-partition chi contribution
     (~8.7us/t at 1x; 4x fewer elements than the cell blocks).
   * ACT t in [TSPLIT,63): Sign(K - t - 0.5)+accum over the two 16384-cell
     blocks (~28.3us/t); host decodes counts from the +/-1 sums.  ACT
     scratch aliases the dead xf input buffer (bitcast) so both engines
     run concurrently.
 - t=63 skipped: chi(63) = 1 exactly (full square); host fills it in.
 - per-slice (2-partition) reduction via PE matmuls with block-ones weights
   into PSUM, split so DVE-owned columns ship while ACT finishes.

Toolchain notes: this container's walrus rejects >1 sync wait per instruction
(_legalize_waits splits them onto NoOps).  tensor_scalar with accum_out is the
TensorScalarPtrReduce variant: op1 is the REDUCTION op (add) and scalar2
combines into the reduced value (0.0).  Plain tensor_scalar/tensor_tensor on
packed bf16 run in DVE 2x perf mode; fused scalar_tensor_tensor runs 1x.
"""

import numpy as np

B, C, H, W = 32, 16, 128, 128
RES = 64
NCORES = 8
SLICES = B * C                  # 512
SPC = SLICES // NCORES          # 64 slices per core
NPART = 128
RPP = 64                        # image rows per partition
XROWS = RPP + 2                 # above aux + owned + below aux
XW = XROWS * W                  # 8448 f32 per partition
VN = RPP * W                    # 8192 cells per class per partition
POSW = 2 * VN                   # 16384: [V|Q] block width
KAUX = 4 * VN                   # below aux row bins (row 64)
EHAUX = 4 * VN + W              # aux Eh row
KBAUX = 4 * VN + 2 * W          # above aux row bins (row -1)
KCAT_W = 4 * VN + 3 * W         # 33152
TSPLIT = 46                     # t in [0,TSPLIT) on DVE (weighted); rest ACT
HYBRID = (62,)                  # V-pass of t=62 moved to DVE tail (plain count)

_CACHE = {}


def _build_program(legalize: bool = True):
    import concourse.bass as bass
    import concourse.mybir as mybir
    from concourse.tile import TileContext
    from concourse.alu_op_type import AluOpType as alu

    dt = mybir.dt
    nc = bass.Bass("TRN2", target_bir_lowering=False, debug=False)

    x_dram = nc.dram_tensor("xi", [NPART, XW], dt.float32, kind="ExternalInput").ap()
    bsel_dram = nc.dram_tensor("bsel", [NPART, SPC], dt.float32, kind="ExternalInput").ap()
    out_dram = nc.dram_tensor("cnt", [SPC, 2 * RES], dt.float32, kind="ExternalOutput").ap()

    # binning chunk xf-row ranges (xf row 0 = above aux, 1..64 = owned, 65 =
    # below).  First chunk small so the compute pipeline starts early.
    chunks = [(0, 8), (8, 24), (24, 40), (40, 56), (56, XROWS)]

    with TileContext(nc) as tc:
        with (
            tc.tile_pool(name="cst", bufs=1) as cpool,
            tc.tile_pool(name="ps", bufs=1, space="PSUM") as pspool,
        ):
            # first (small) chunk DMA issued before anything else so the
            # binning pipeline starts as early as possible
            xf = cpool.tile([NPART, XW], dt.float32)
            r0, r1 = chunks[0]
            nc.sync.dma_start(xf[:, r0 * W : r1 * W], x_dram[:, r0 * W : r1 * W])
            blocksel = cpool.tile([NPART, SPC], dt.float32)
            nc.sync.dma_start(blocksel[:, :], bsel_dram)
            for r0, r1 in chunks[1:]:
                nc.sync.dma_start(xf[:, r0 * W : r1 * W], x_dram[:, r0 * W : r1 * W])

            # big scratch declared bf16 NATIVELY: DVE perf modes check the
            # underlying tile dtype, so bf16 slices here run 2x while the
            # f32/int32 binning views (1x ops regardless) are bitcasts.
            big = cpool.tile([NPART, 8 * 17 * W], dt.bfloat16)   # 17408 bf16
            ybuf = [
                big[:, 0 : 2 * 17 * W].bitcast(dt.float32),
                big[:, 2 * 17 * W : 4 * 17 * W].bitcast(dt.float32),
            ]
            ki = big[:, 4 * 17 * W : 6 * 17 * W].bitcast(dt.int32)
            yt = big[:, 6 * 17 * W : 7 * 17 * W]
            mg = big[:, 7 * 17 * W : 8 * 17 * W]

            kcat = cpool.tile([NPART, KCAT_W], dt.bfloat16)
            # weight-compute scratch arrays ([65,127] packed rows fit in 8320)
            wk1 = cpool.tile([NPART, 8320], dt.bfloat16)   # doubles as DVE scr
            wk2 = cpool.tile([NPART, 8320], dt.bfloat16)
            wk4 = big[:, 0:8320]                           # real bf16 slices
            wk5 = big[:, 8704:17024]
            wgt = cpool.tile([NPART, VN], dt.bfloat16)
            ehm1 = cpool.tile([NPART, W], dt.bfloat16)

            bias = cpool.tile([NPART, RES], dt.float32)
            for t in range(TSPLIT, RES - 1):
                nc.vector.memset(bias[:, t : t + 1], -(t + 0.5))
            cnt = cpool.tile([NPART, 2 * RES], dt.float32)
            nc.vector.memset(cnt[:, RES - 1 : RES], 0.0)
            nc.vector.memset(cnt[:, 2 * RES - 1 : 2 * RES], 0.0)

            # ---- exact binning into kcat [kbaux | V rows | kaux] ----
            # ACT casts straight from xf (ki = int(63*x), yt = bf16(ki)) run
            # independently of DVE's muls (y = 63*x, double-buffered), so the
            # two engines pipeline without stalls.  K = yt + (y > yt) is
            # exact for any cast landing in {floor(y), ceil(y)}; yt/m bf16
            # (integers <= 126 exact) so the combine adds run 2x.
            for c, (r0, r1) in enumerate(chunks):
                n = (r1 - r0) * W
                xc = xf[:, r0 * W : r1 * W]
                y = ybuf[c % 2]
                nc.vector.tensor_scalar_mul(y[:, 0:n], xc, 63.0)
                nc.scalar.activation(
                    ki[:, 0:n], xc, mybir.ActivationFunctionType.Copy,
                    bias=0.0, scale=63.0,
                )
                nc.scalar.activation(
                    yt[:, 0:n], ki[:, 0:n], mybir.ActivationFunctionType.Copy,
                    bias=0.0, scale=1.0,
                )
                nc.vector.tensor_tensor(
                    mg[:, 0:n], y[:, 0:n], yt[:, 0:n], alu.is_gt
                )
                # dest list: xf row 0 -> kbaux, rows 1..64 -> V rows 0..63,
                # row 65 -> kaux
                dsts = []
                lo = 0
                if r0 == 0:
                    dsts.append((kcat[:, KBAUX : KBAUX + W], 0, W))
                    lo = W
                hi = min(r1, RPP + 1) * W - r0 * W
                if hi > lo:
                    dsts.append(
                        (kcat[:, (r0 - 1) * W + lo : (min(r1, RPP + 1) - 1) * W], lo, hi)
                    )
                if r1 == XROWS:
                    dsts.append((kcat[:, KAUX : KAUX + W], hi, hi + W))
                for dst, a, b in dsts:
                    nc.vector.tensor_tensor(dst, yt[:, a:b], mg[:, a:b], alu.add)

            k3 = kcat[:, 0:VN].rearrange("p (r w) -> p r w", w=W)
            q3 = kcat[:, VN : 2 * VN].rearrange("p (r w) -> p r w", w=W)
            eh3 = kcat[:, 2 * VN : 3 * VN].rearrange("p (r w) -> p r w", w=W)
            ev3 = kcat[:, 3 * VN : 4 * VN].rearrange("p (r w) -> p r w", w=W)
            kax = kcat[:, KAUX : KAUX + W]
            ehaux = kcat[:, EHAUX : EHAUX + W]
            kbx = kcat[:, KBAUX : KBAUX + W]

            # ---- neighbor maxes for the ACT cell blocks (bf16, 2x) ----
            nc.vector.tensor_tensor(
                eh3[:, :, 0 : W - 1], k3[:, :, 0 : W - 1], k3[:, :, 1:W], alu.max
            )
            nc.vector.memset(eh3[:, :, W - 1 : W], 64.0)
            nc.vector.tensor_tensor(
                ehaux[:, 0 : W - 1], kax[:, 0 : W - 1], kax[:, 1:W], alu.max
            )
            # Ev/Q maxes on the otherwise-idle Pool engine: their consumers
            # (ACT's neg/Q passes, the D weight op) have large slack, so
            # Pool's slower Q7 rate is hidden.  Q first (D needs it sooner).
            nc.vector.tensor_tensor(
                ev3[:, 0 : RPP - 1, :], k3[:, 0 : RPP - 1, :], k3[:, 1:RPP, :], alu.max
            )
            nc.vector.tensor_tensor(
                ev3[:, RPP - 1, :], k3[:, RPP - 1, :], kax[:, :], alu.max
            )
            nc.vector.tensor_tensor(
                q3[:, 0 : RPP - 1, 0 : W - 1],
                eh3[:, 0 : RPP - 1, 0 : W - 1], eh3[:, 1:RPP, 0 : W - 1], alu.max,
            )
            nc.vector.tensor_tensor(
                q3[:, RPP - 1, 0 : W - 1],
                eh3[:, RPP - 1, 0 : W - 1], ehaux[:, 0 : W - 1], alu.max,
            )
            nc.vector.memset(q3[:, :, W - 1 : W], 64.0)

            # ---- ACT threshold loop.  Sign(K - t - 0.5) sums, one pass per
            # cell class so each starts as soon as its data is ready:
            # V-block passes first (ready right after binning, before any
            # maxes -- fills ACT's wait-for-maxes gap), then neg [Eh|Ev],
            # then Q.  Sums land in separate cnt columns:
            #   S_V -> cnt[64 + i], S_Q -> cnt[81 + i] (i = t - TSPLIT),
            #   S_neg -> cnt[RES + t];  host: chi = (S_neg - S_V - S_Q)/2.
            scra = xf[:, 0:VN].bitcast(mybir.dt.bfloat16)
            for t in range(TSPLIT, RES - 1):
                if t in HYBRID:
                    continue        # V-pass done on DVE (plain count) instead
                i = RES + (t - TSPLIT)
                nc.scalar.activation(
                    scra[:, 0:VN], kcat[:, 0:VN],
                    mybir.ActivationFunctionType.Sign,
                    bias=bias[:, t : t + 1], scale=1.0,
                    accum_out=cnt[:, i : i + 1],
                )
            for t in range(TSPLIT, RES - 1):
                nc.scalar.activation(
                    scra[:, :], kcat[:, POSW : 2 * POSW],
                    mybir.ActivationFunctionType.Sign,
                    bias=bias[:, t : t + 1], scale=1.0,
                    accum_out=cnt[:, RES + t : RES + t + 1],
                )
            for t in range(TSPLIT, RES - 1):
                i = RES + 17 + (t - TSPLIT)
                nc.scalar.activation(
                    scra[:, 0:VN], kcat[:, VN : 2 * VN],
                    mybir.ActivationFunctionType.Sign,
                    bias=bias[:, t : t + 1], scale=1.0,
                    accum_out=cnt[:, i : i + 1],
                )

            # ---- lower-star vertex weights (DVE).  Storage row s = grid row
            # s-1 relative to the partition strip (s=0 is the above-aux row).
            WJ = W - 1  # 127
            Hm = wk1[:, 0 : 65 * WJ].rearrange("p (r w) -> p r w", w=WJ)
            Ve = wk2[:, 0 : 65 * W].rearrange("p (r w) -> p r w", w=W)
            wg = wgt[:, :].rearrange("p (r w) -> p r w", w=W)

            # He: [k(r,j+1) >= k(r,j)]
            nc.vector.tensor_tensor(Hm[:, 0, :], kbx[:, 1:W], kbx[:, 0:WJ], alu.is_ge)
            nc.vector.tensor_tensor(
                Hm[:, 1:65, :], k3[:, :, 1:W], k3[:, :, 0:WJ], alu.is_ge
            )
            # Ve[s] = [row(s) >= row(s-1)]  (v-edge between rows s-1, s)
            nc.vector.tensor_tensor(Ve[:, 0, :], k3[:, 0, :], kbx[:, :], alu.is_ge)
            nc.vector.tensor_tensor(
                Ve[:, 1:64, :], k3[:, 1:RPP, :], k3[:, 0 : RPP - 1, :], alu.is_ge
            )
            nc.vector.tensor_tensor(Ve[:, 64, :], kax[:, :], k3[:, RPP - 1, :], alu.is_ge)
            # w init: Ve(below) - Ve(above).  The "+1 vertex - 1 right-edge
            # - 1 below-edge" constants fold into: +1-1 = 0 for cols 0..126
            # handled by flipping the A term (w += A-1 = -[not A]) below, and
            # the remaining +1-1 = 0 via Ve complements baked into this form.
            nc.vector.tensor_tensor(
                wg[:, :, :], Ve[:, 1:65, :], Ve[:, 0:64, :], alu.subtract
            )
            # w += He(i,j) (right-edge assigned right); w -= He(i,j-1) (left)
            nc.vector.tensor_tensor(
                wg[:, :, 0:WJ], wg[:, :, 0:WJ], Hm[:, 1:65, :], alu.add
            )
            nc.vector.tensor_tensor(
                wg[:, :, 1:W], wg[:, :, 1:W], Hm[:, 1:65, :], alu.subtract
            )
            # squares between rows s, s+1 (s = 0..64): corners
            # a=(s,j) b=(s,j+1) c=(s+1,j) d=(s+1,j+1); Eh over row s needed.
            # CE first (doesn't need Pool's q3; buys the Pool Q ops slack).
            nc.vector.tensor_tensor(ehm1[:, 0:WJ], kbx[:, 0:WJ], kbx[:, 1:W], alu.max)
            CE = wk4[:, 0 : 65 * WJ].rearrange("p (r w) -> p r w", w=WJ)
            nc.vector.tensor_tensor(CE[:, 0, :], k3[:, 0, 0:WJ], ehm1[:, 0:WJ], alu.is_ge)
            nc.vector.tensor_tensor(
                CE[:, 1:64, :], k3[:, 1:RPP, 0:WJ], eh3[:, 0:63, 0:WJ], alu.is_ge
            )
            nc.vector.tensor_tensor(
                CE[:, 64, :], kax[:, 0:WJ], eh3[:, RPP - 1, 0:WJ], alu.is_ge
            )
            cwt = cpool.tile([NPART, 65 * WJ], dt.bfloat16)
            mx0 = cwt[:, 0:WJ]
            nc.vector.tensor_tensor(mx0, ehm1[:, 0:WJ], k3[:, 0, 0:WJ], alu.max)
            Dw = wk5[:, 0 : 65 * WJ].rearrange("p (r w) -> p r w", w=WJ)
            # d wins iff k_d >= max(a,b,c) iff k_d >= cellmax = q3 (reused!)
            nc.vector.tensor_tensor(Dw[:, 0, :], k3[:, 0, 1:W], mx0, alu.is_ge)
            nc.vector.tensor_tensor(
                Dw[:, 1:64, :], k3[:, 1:RPP, 1:W], q3[:, 0:63, 0:WJ], alu.is_ge
            )
            nc.vector.tensor_tensor(
                Dw[:, 64, :], kax[:, 1:W], q3[:, RPP - 1, 0:WJ], alu.is_ge
            )
            Cw = cwt[:, :].rearrange("p (r w) -> p r w", w=WJ)
            nc.vector.tensor_tensor(Cw[:, :, :], CE[:, :, :], Dw[:, :, :], alu.is_gt)
            # w += C(i-1, j): vertex row i <- square row s=i, cols 0..126
            nc.vector.tensor_tensor(
                wg[:, :, 0:WJ], wg[:, :, 0:WJ], Cw[:, 0:64, :], alu.add
            )
            DC = wk4[:, 0 : 65 * WJ].rearrange("p (r w) -> p r w", w=WJ)  # CE dead
            nc.vector.tensor_tensor(DC[:, :, :], Dw[:, :, :], Cw[:, :, :], alu.add)
            # w += D(i-1, j-1): square row s=i, shifted right
            nc.vector.tensor_tensor(
                wg[:, :, 1:W], wg[:, :, 1:W], Dw[:, 0:64, :], alu.add
            )
            Bb = wk5[:, 0 : 64 * WJ].rearrange("p (r w) -> p r w", w=WJ)  # Dw dead
            nc.vector.tensor_tensor(
                Bb[:, :, :], Hm[:, 1:65, :], DC[:, 1:65, :], alu.is_gt
            )
            nc.vector.tensor_tensor(
                wg[:, :, 1:W], wg[:, :, 1:W], Bb[:, :, :], alu.add
            )
            DCH = cwt[:, 0 : 64 * WJ].rearrange("p (r w) -> p r w", w=WJ)  # Cw dead
            nc.vector.tensor_tensor(
                DCH[:, :, :], DC[:, 1:65, :], Hm[:, 1:65, :], alu.add
            )
            # w += A - 1 on cols 0..126 (the folded right/below-edge consts):
            # A-1 = -[not A] = -[DCH >= 0.5]
            NA = wk4[:, 0 : 64 * WJ].rearrange("p (r w) -> p r w", w=WJ)  # DC dead
            nc.vector.tensor_scalar(NA[:, :, :], DCH[:, :, :], 0.5, None, alu.is_ge)
            nc.vector.tensor_tensor(
                wg[:, :, 0:WJ], wg[:, :, 0:WJ], NA[:, :, :], alu.subtract
            )

            # ---- threshold loops, DVE and ACT fully overlapped ----
            # DVE: fused weighted vertex count: chi(t) directly in cnt[:, t].
            scr8 = wk1[:, 0:VN]
            for t in range(TSPLIT):
                nc.vector.scalar_tensor_tensor(
                    scr8, kcat[:, 0:VN], float(t), wgt[:, :],
                    alu.is_le, alu.mult,
                    accum_out=cnt[:, t : t + 1],
                )
            # hybrid: plain V count on DVE's tail (ACT slack), same column
            # the ACT V-pass would have used; host converts to Sign form
            for t in HYBRID:
                i = RES + (t - TSPLIT)
                nc.vector.tensor_scalar(
                    scr8, kcat[:, 0:VN], float(t), 0.0, alu.is_le, alu.add,
                    accum_out=cnt[:, i : i + 1],
                )
            # ---- per-slice (2-partition) reduction on PE, split so the
            # DVE-owned columns ship while ACT finishes its last passes ----
            psum = pspool.tile([SPC, 2 * RES], dt.float32)
            outt = cpool.tile([SPC, 2 * RES], dt.float32)
            nc.tensor.matmul(
                psum[:, 0:TSPLIT], blocksel[:, 0:SPC], cnt[:, 0:TSPLIT],
                start=True, stop=True,
            )
            nc.vector.tensor_copy(outt[:, 0:TSPLIT], psum[:, 0:TSPLIT])
            nc.sync.dma_start(out_dram[:, 0:TSPLIT], outt[:, 0:TSPLIT])
            nc.tensor.matmul(
                psum[:, TSPLIT:], blocksel[:, 0:SPC], cnt[:, TSPLIT:],
                start=True, stop=True,
            )
            nc.vector.tensor_copy(outt[:, TSPLIT:], psum[:, TSPLIT:])
            nc.sync.dma_start(out_dram[:, TSPLIT:], outt[:, TSPLIT:])

    if legalize:
        _legalize_waits(nc)
    return nc


def _legalize_waits(nc, max_waits: int = 1):
    """This walrus build rejects instructions with more than one sync wait.
    Split excess waits onto preceding same-engine NoOps."""
    import concourse.mybir as mybir

    for f in nc.m.functions:
        for b in f.blocks:
            il = list(b.instructions)
            out, changed = [], False
            for inst in il:
                try:
                    si = inst.sync_info
                except AttributeError:
                    si = None
                waits = list(si.on_wait) if si else []
                if len(waits) > max_waits:
                    head, keep = waits[:-max_waits], waits[-max_waits:]
                    for k, w in enumerate(head):
                        out.append(
                            mybir.InstNoOp(
                                name=f"{inst.name}-w{k}",
                                engine=inst.engine,
                                sync_info=mybir.SyncInfo(on_wait=[w], on_update=[]),
                                bass_nofuse=True,
                            )
                        )
                    inst.sync_info = mybir.SyncInfo(
                        on_wait=keep, on_update=list(si.on_update)
                    )
                    changed = True
                out.append(inst)
            if changed:
                b.instructions = out


def make_host_inputs(xcore: np.ndarray):
    """xcore [SPC*H, W] f32 -> (xi, bsel) host-side input arrays."""
    r = xcore.reshape(NPART, RPP, W)
    xi = np.empty((NPART, XROWS, W), dtype=np.float32)
    xi[:, 1 : RPP + 1] = r
    # above aux row (grid row -1): real for odd partitions (mid-slice)
    xi[:, 0] = 2.0
    xi[1::2, 0] = r[0::2, RPP - 1]
    # below aux row (grid row 64): real for even partitions
    xi[:, RPP + 1] = 2.0
    xi[0:NPART - 1 : 2, RPP + 1] = r[1::2, 0]
    bsel = np.zeros((NPART, SPC), dtype=np.float32)
    bsel[np.arange(NPART), np.arange(NPART) // 2] = 1.0
    return xi.reshape(NPART, XW), bsel


def _install_ntff_hook():
    """Provide antenv.axon_hooks (absent in this container) so
    run_bass_kernel_spmd(trace=True) can capture NTFF profiles."""
    import sys, types

    if "antenv.axon_hooks" in sys.modules:
        return
    mod = types.ModuleType("antenv.axon_hooks")
    state = {"hook": None}
    mod.set_axon_ntff_profile_hook = lambda h: state.update(hook=h)
    mod.get_axon_ntff_profile_hook = lambda: state["hook"]
    sys.modules["antenv.axon_hooks"] = mod
    try:
        from trn_agent_boot.trn_boot import _ntff_profile_via_ctypes

        hook = _ntff_profile_via_ctypes("/opt/axon/libaxon_pjrt.so")
        if hook is not None:
            mod.set_axon_ntff_profile_hook(hook)
    except Exception:
        pass


def _run(x: np.ndarray, trace: bool = False):
    from concourse import bass_utils

    if trace:
        _install_ntff_hook()

    x = np.ascontiguousarray(np.asarray(x), dtype=np.float32)
    assert x.shape == (B, C, H, W)

    if "nc" not in _CACHE:
        _CACHE["nc"] = _build_program()
    nc = _CACHE["nc"]

    flat = x.reshape(NCORES, SPC * H, W)
    in_maps = []
    for k in range(NCORES):
        xi, bsel = make_host_inputs(flat[k])
        in_maps.append({"xi": xi, "bsel": bsel})
    res = bass_utils.run_bass_kernel_spmd(
        nc, in_maps, core_ids=list(range(NCORES)), trace=trace
    )
    outs = [r["cnt"] for r in res.results]  # each [SPC, 2*RES] f32
    cnt = np.stack(outs, axis=0).reshape(SLICES, 2 * RES)
    ecc = np.empty((SLICES, RES), dtype=np.float32)
    # DVE thresholds: weighted vertex counts = chi directly
    ecc[:, 0:TSPLIT] = cnt[:, 0:TSPLIT]
    # ACT thresholds: per-class Sign sums, chi = (S_neg - S_V - S_Q) / 2.
    # Hybrid thresholds: the V column holds a plain DVE count C_V instead;
    # S_V = 2*VN*2partitions - 2*C_V = POSW - 2*C_V... per-slice V slots =
    # 16384 = POSW, so chi = (S_neg - S_Q - POSW + 2*C_V) / 2.
    for t in range(TSPLIT, RES - 1):
        i = t - TSPLIT
        if t in HYBRID:
            ecc[:, t] = (
                cnt[:, RES + t] - cnt[:, RES + 17 + i]
                + 2.0 * cnt[:, RES + i] - POSW
            ) / 2.0
        else:
            ecc[:, t] = (
                cnt[:, RES + t] - cnt[:, RES + i] - cnt[:, RES + 17 + i]
            ) / 2.0
    # chi at the top threshold is the Euler characteristic of the full square
    ecc[:, RES - 1] = 1.0
    return ecc.reshape(B, C, RES).astype(np.float32), res


def kernel(x: np.ndarray) -> np.ndarray:
    out, _ = _run(x, trace=False)
    return out
